# revision 1
# baseline (speedup 1.0000x reference)
"""Trainium2 Bass kernel for nn_CrossRPEAttentionMulti.

Sharding: 8 cores = batch(4) x head-group(2). Each core computes, for its
(b, g): kT = Wk_g @ x_b^T, V = x_b @ Wv_g^T, transposed attention
S^T = kT^T-slices @ qT with RPE bias added on the key axis, exp (no max
subtraction -- logits are bounded ~|3|), unnormalized out^T = V_aug^T @ P^T
with an appended ones-column producing the softmax denominators, per-head
normalization via reciprocal + partition-broadcast, and the output
projection y_partial = out_norm^T^T @ Wp_g^T. Host sums the two group
partials per batch and adds the bias.

All matmuls run in float32r (1 cyc/row on the PE for free-dim >= 256,
~1.5e-4 relative error). The RPE bias table is precomputed on the host
(tiny: <0.1% of FLOPs), expanded to the key-tile layout, and streamed as
bf16.
"""
import numpy as np
import ml_dtypes

import concourse.mybir as mybir
import concourse.tile as tile
from concourse import bacc
from concourse.bass_utils import run_bass_kernel_spmd

f32 = mybir.dt.float32
f32r = mybir.dt.float32r
bf16 = mybir.dt.bfloat16

# -- static problem configuration (matches the reference module) --
B, C, H, G = 4, 1024, 16, 24
P_SP = G * G            # 576 spatial patches / modality
LQ = P_SP + 1           # 577 queries
NKV = 3 * P_SP + 1      # 1729 keys/values
HD = C // H             # 64
HPC = 8                 # heads per core (16 heads / 2 groups)
NCORES = 8

NPAD = 1792             # keys padded to 14*128
NT = NPAD // 128        # 14 key tiles
QPAD = 768              # queries padded to 512+256 (both chunks >=256)
NBLOCKS = [(0, 512), (512, 1024), (1024, 1536), (1536, 1792)]
VSTRIDE = 66            # per-head V cols: 64 dims + ones col + pad (fp32r needs even M)


def _build_nc():
    nc = bacc.Bacc("TRN2", target_bir_lowering=False, debug=False)

    import os
    dbg = os.environ.get("KDEBUG") == "1"
    xT = nc.dram_tensor("xT", [C, NPAD], f32r, kind="ExternalInput")
    wkT = nc.dram_tensor("wkT", [C, 512], f32r, kind="ExternalInput")
    wvT = nc.dram_tensor("wvT", [C, 512], f32r, kind="ExternalInput")
    wpT = nc.dram_tensor("wpT", [512, C], f32r, kind="ExternalInput")
    qT = nc.dram_tensor("qT", [128, 4, QPAD], f32r, kind="ExternalInput")
    biasx = nc.dram_tensor("biasx", [HPC, NT, 128, LQ], bf16, kind="ExternalInput")
    y = nc.dram_tensor("y", [LQ, C], f32, kind="ExternalOutput")
    if dbg:
        d_kT = nc.dram_tensor("d_kT", [128, 4, NPAD], f32r, kind="ExternalOutput")
        d_v = nc.dram_tensor("d_v", [128, NT, HPC * VSTRIDE], f32r, kind="ExternalOutput")
        d_rec = nc.dram_tensor("d_rec", [128, 4, LQ + 1], f32, kind="ExternalOutput")
        d_outT = nc.dram_tensor("d_outT", [128, 4, LQ + 1], f32r, kind="ExternalOutput")

    xTr = xT.rearrange("(j p) n -> j p n", p=128)

    with tile.TileContext(nc) as tc:
        with (
            tc.tile_pool(name="main", bufs=1) as main,
            tc.tile_pool(name="ptp", bufs=3) as ptp,
            tc.tile_pool(name="biasp", bufs=4) as biasp,
            tc.tile_pool(name="recp", bufs=2) as recp,
            tc.tile_pool(name="yp", bufs=2) as yp,
        ):
            kT_sb = main.tile([128, 4, NPAD], f32r)
            v_sb = main.tile([128, NT, HPC * VSTRIDE], f32r)
            qT_sb = main.tile([128, 4, QPAD], f32r)
            wpT_sb = main.tile([128, 4, C], f32r)
            outT = main.tile([128, 4, LQ + 1], f32r)
            rec_full = main.tile([128, 4, LQ + 1], f32)
            nc.vector.memset(rec_full[:, :, LQ:LQ + 1], 0.0)
            nc.vector.tensor_copy(outT[:, :, LQ:LQ + 1], rec_full[:, :, LQ:LQ + 1])

            nc.sync.dma_start(qT_sb, qT.ap())
            nc.sync.dma_start(wpT_sb, wpT.rearrange("(j p) n -> p j n", p=128))

            # ones column of V_aug (gives softmax denominators for free);
            # t=13 rows 65.. are x-padding -> keep their ones at 0.
            vre = v_sb.rearrange("p t (h e) -> p t h e", e=VSTRIDE)
            ones_f = main.tile([128, NT, HPC, 2], f32)
            nc.vector.memset(ones_f[:, :, :, 1], 0.0)
            nc.vector.memset(ones_f[:, 0:13, :, 0], 1.0)
            nc.vector.memset(ones_f[64:128, 13, :, 0], 0.0)
            nc.vector.memset(ones_f[64:65, 13, :, 0], 1.0)
            nc.vector.memset(ones_f[0:64, 13, :, 0], 1.0)
            nc.vector.tensor_copy(vre[:, :, :, 64:66], ones_f)

            # ---- phase 1+2: kT and V, streaming x^T blocks ----
            with (
                tc.tile_pool(name="wk", bufs=1) as wk,
                tc.tile_pool(name="xs", bufs=2) as xs,
                tc.tile_pool(name="psmm", bufs=3, space="PSUM") as psmm,
            ):
                wkT_sb = wk.tile([128, 8, 512], f32r)
                wvT_sb = wk.tile([128, 8, 512], f32r)
                nc.sync.dma_start(wkT_sb, wkT.rearrange("(j p) m -> p j m", p=128))
                nc.sync.dma_start(wvT_sb, wvT.rearrange("(j p) m -> p j m", p=128))

                for bi, (n0, n1) in enumerate(NBLOCKS):
                    w = n1 - n0
                    xblk = xs.tile([128, 8, 512], f32r, tag="xblk")
                    for kj in range(8):
                        nc.sync.dma_start(xblk[:, kj, 0:w], xTr[kj][:, n0:n1])
                    # kT rows for this n-block (all 4 c'-tiles)
                    for mt in range(4):
                        ps = psmm.tile([128, 512], f32, tag="ps")
                        for kj in range(8):
                            nc.tensor.matmul(
                                ps[:, 0:w],
                                wkT_sb[:, kj, mt * 128:(mt + 1) * 128],
                                xblk[:, kj, 0:w],
                                start=(kj == 0), stop=(kj == 7),
                            )
                        nc.vector.tensor_copy(kT_sb[:, mt, n0:n1], ps[:, 0:w])
                    # V tiles inside this n-block
                    for t in range(bi * 4, min(bi * 4 + 4, NT)):
                        rel = t * 128 - n0
                        ps = psmm.tile([128, 512], f32, tag="ps")
                        for kj in range(8):
                            nc.tensor.matmul(
                                ps,
                                xblk[:, kj, rel:rel + 128],
                                wvT_sb[:, kj, :],
                                start=(kj == 0), stop=(kj == 7),
                            )
                        nc.scalar.copy(
                            vre[:, t, :, 0:64],
                            ps.rearrange("p (h e) -> p h e", e=64),
                        )

            # ---- phase 3: attention ----
            with (
                tc.tile_pool(name="psst", bufs=2, space="PSUM") as psst,
                tc.tile_pool(name="psout", bufs=2, space="PSUM") as psout,
            ):
                for h in range(HPC):
                    pb = (h % 2) * 64
                    j = h // 2
                    ops = psout.tile([66, LQ + 1], f32, tag="ops")
                    for t in range(NT):
                        bt = biasp.tile([128, LQ], bf16, tag="bt")
                        nc.sync.dma_start(bt, biasx.ap()[h, t])
                        st = psst.tile([128, QPAD], f32, tag="st")
                        lk = kT_sb[pb:pb + 64, j, t * 128:(t + 1) * 128]
                        for (q0, q1) in ((0, 512), (512, QPAD)):
                            nc.tensor.matmul(
                                st[:, q0:q1], lk, qT_sb[pb:pb + 64, j, q0:q1],
                                start=True, stop=True,
                            )
                        nc.vector.tensor_add(
                            out=st[:, 0:LQ], in0=st[:, 0:LQ], in1=bt)
                        pt = ptp.tile([128, LQ + 1], f32r, tag="pt")
                        nc.scalar.activation(
                            pt, st[:, 0:LQ + 1], mybir.ActivationFunctionType.Exp)
                        lv = v_sb[:, t, h * VSTRIDE:(h + 1) * VSTRIDE]
                        for (q0, q1) in ((0, 512), (512, LQ + 1)):
                            nc.tensor.matmul(
                                ops[:, q0:q1], lv, pt[:, q0:q1],
                                start=(t == 0), stop=(t == NT - 1),
                            )
                    rec = recp.tile([1, LQ], f32, tag="rec")
                    nc.vector.reciprocal(rec, ops[64:65, 0:LQ])
                    rbc = recp.tile([64, LQ], f32, tag="rbc")
                    nc.gpsimd.partition_broadcast(rbc, rec)
                    nc.vector.tensor_copy(rec_full[pb:pb + 64, j, 0:LQ], rbc)
                    nc.vector.tensor_copy(outT[pb:pb + 64, j, 0:LQ], ops[0:64, 0:LQ])

            # ---- phase 4: normalize + projection ----
            with tc.tile_pool(name="pspj", bufs=2, space="PSUM") as pspj:
                if dbg:
                    nc.sync.dma_start(d_kT.ap(), kT_sb)
                    nc.sync.dma_start(d_v.ap(), v_sb)
                    nc.sync.dma_start(d_rec.ap(), rec_full)
                    nc.sync.dma_start(d_outT.ap(), outT)
                nc.vector.tensor_mul(out=outT, in0=outT, in1=rec_full)
                for mt in range(5):
                    m0 = mt * 128
                    mcols = 66 if mt == 4 else 128   # lhsT free width (even)
                    mrows = 65 if mt == 4 else 128   # valid output rows
                    ps = pspj.tile([128, C], f32, tag="pp")
                    for j in range(4):
                        for (c0, c1) in ((0, 512), (512, C)):
                            nc.tensor.matmul(
                                ps[:mcols, c0:c1],
                                outT[:, j, m0:m0 + mcols],
                                wpT_sb[:, j, c0:c1],
                                start=(j == 0), stop=(j == 3),
                            )
                    for (c0, c1) in ((0, 512), (512, C)):
                        yt = yp.tile([128, 512], f32, tag="yt")
                        nc.vector.tensor_copy(yt[:mrows], ps[:mrows, c0:c1])
                        nc.sync.dma_start(y.ap()[m0:m0 + mrows, c0:c1], yt[:mrows])

    nc.finalize()
    return nc


_NC_CACHE = None


def _get_nc():
    global _NC_CACHE
    if _NC_CACHE is None:
        _NC_CACHE = _build_nc()
    return _NC_CACHE


def _host_prep(x, q_learned, pos_embed, Wk, Wv, Wp, rpe_W, rp_bucket):
    """Build the 8 per-core input maps."""
    x = np.asarray(x, dtype=np.float32)
    q_ = (np.asarray(q_learned, np.float32) + np.asarray(pos_embed, np.float32))[0]
    Wk = np.asarray(Wk, np.float32)
    Wv = np.asarray(Wv, np.float32)
    Wp = np.asarray(Wp, np.float32)
    rpe_W = np.asarray(rpe_W, np.float32)
    rp_bucket = np.asarray(rp_bucket)

    scale = HD ** -0.5

    # RPE bias, expanded to key-tile layout, transposed: biasx[h, t, p, q]
    qh = q_.reshape(LQ, H, HD)
    rpe_tab = np.einsum('qhd,dn->hqn', qh, rpe_W)                  # (H, LQ, nb)
    rpe = np.take_along_axis(
        rpe_tab, np.broadcast_to(rp_bucket[None], (H, LQ, LQ)), axis=-1
    )                                                              # (H, q, j')
    n_idx = np.arange(NPAD)
    jcol = np.where(n_idx == 0, 0, 1 + (n_idx - 1) % P_SP)         # (NPAD,)
    biasx = rpe[:, :, jcol]                                        # (H, q, n)
    biasx[:, :, NKV:] = 0.0
    biasx = np.ascontiguousarray(
        biasx.transpose(0, 2, 1)                                   # (H, n, q)
    ).reshape(H, NT, 128, LQ).astype(ml_dtypes.bfloat16)

    # qT per group, scaled, padded: (2, 128, 4, QPAD)
    qTg = np.zeros((2, 512, QPAD), np.float32)
    qTg[:, :, :LQ] = (q_.T * scale).reshape(2, 512, LQ)
    qTg = qTg.reshape(2, 4, 128, QPAD).transpose(0, 2, 1, 3).copy()

    per_group = []
    for g in range(2):
        sl = slice(g * 512, (g + 1) * 512)
        per_group.append({
            "wkT": np.ascontiguousarray(Wk[sl, :].T),
            "wvT": np.ascontiguousarray(Wv[sl, :].T),
            "wpT": np.ascontiguousarray(Wp[:, sl].T),
            "qT": np.ascontiguousarray(qTg[g]),
            "biasx": np.ascontiguousarray(biasx[g * HPC:(g + 1) * HPC]),
        })

    in_maps = []
    for b in range(B):
        xTb = np.zeros((C, NPAD), np.float32)
        xTb[:, :NKV] = x[b].T
        for g in range(2):
            m = dict(per_group[g])
            m["xT"] = xTb
            in_maps.append(m)
    return in_maps


def kernel(x, q_learned, pos_embed, Wk, Wv, Wp, bp, rpe_W, rp_bucket):
    in_maps = _host_prep(x, q_learned, pos_embed, Wk, Wv, Wp, rpe_W, rp_bucket)
    nc = _get_nc()

    last_err = None
    for _attempt in range(3):
        try:
            res = run_bass_kernel_spmd(nc, in_maps, core_ids=list(range(NCORES)))
            break
        except Exception as e:  # wedged-device recovery: retry
            last_err = e
    else:
        raise last_err

    bp = np.asarray(bp, np.float32)
    out = np.empty((B, LQ, C), np.float32)
    for b in range(B):
        out[b] = res.results[2 * b]["y"] + res.results[2 * b + 1]["y"] + bp
    return out



# revision 2
# speedup vs baseline: 52956.0498x; 52956.0498x over previous
"""Trainium2 Bass kernel for nn_CrossRPEAttentionMulti — v8.

Sharding: 8 cores = batch(4) x head-group(2). Per core: one batch, 8 heads.

All matmul operands bf16 (fp32 PSUM accumulation). Bias is applied
multiplicatively after exp (exp(S+b) = exp(S)*exp(b), exp(b) host-side).

Merged pipeline: per x^T block: kT(block) -> QK for all 4 head pairs
(row-tiled head-pair matmuls, adjacent) -> exp on ACT straight from PSUM
-> bf16 multiply by exp(bias) on DVE -> V(block). kv of the next block
overlaps the exps. The LAST block's QK runs after the kv PSUM pool is
closed, so the AV sweeps interleave with its exp tail (PE stays dense).
AV per head (ones column gives denominators), then reciprocal +
partition-broadcast + normalize overlap the next AV sweep. Projection
row-tiles mt0/mt1 accumulate per pair inside the AV loop reusing the QK
PSUM slots; mt2-4 run at the end. Host sums the group partials + bias.
"""
import numpy as np
import ml_dtypes

import concourse.mybir as mybir
import concourse.tile as tile
from concourse import bacc
from concourse.bass_utils import run_bass_kernel_spmd

f32 = mybir.dt.float32
bf16 = mybir.dt.bfloat16

# -- static problem configuration (matches the reference module) --
B, C, H, G = 4, 1024, 16, 24
P_SP = G * G            # 576 spatial patches / modality
LQ = P_SP + 1           # 577 queries
NKV = 3 * P_SP + 1      # 1729 keys/values
HD = C // H             # 64
HPC = 8                 # heads per core (16 heads / 2 groups)
NPAIR = 4               # head pairs per core
NCORES = 8

NPAD = 1792             # keys padded to 14*128
NT = NPAD // 128        # 14 key tiles
QF = LQ + 1             # query free width 578 (even pad)
NBLK = 7                # x^T blocks of 256
VSTRIDE = 66            # per-head V cols: 64 dims + ones col + pad
QCHUNKS = ((0, 512), (512, QF))


def _build_nc():
    nc = bacc.Bacc("TRN2", target_bir_lowering=False, debug=False)

    xT = nc.dram_tensor("xT", [C, NPAD], bf16, kind="ExternalInput")
    wkT = nc.dram_tensor("wkT", [C, 512], bf16, kind="ExternalInput")
    wvT = nc.dram_tensor("wvT", [C, 512], bf16, kind="ExternalInput")
    wpT = nc.dram_tensor("wpT", [512, C], bf16, kind="ExternalInput")
    qT = nc.dram_tensor("qT", [128, 4, QF], bf16, kind="ExternalInput")
    # exp(bias), pair-major: [pair, t, partition, head-in-pair, query]
    biasx = nc.dram_tensor("biasx", [NPAIR, NT, 128, 2, LQ], bf16,
                           kind="ExternalInput")
    y = nc.dram_tensor("y", [5, 128, C], bf16, kind="ExternalOutput")

    xTr = xT.rearrange("(j p) n -> p j n", p=128)

    with tile.TileContext(nc) as tc:
        with (
            tc.tile_pool(name="main", bufs=1) as main,
            tc.tile_pool(name="biasp", bufs=3) as biasp,
            tc.tile_pool(name="recp", bufs=2) as recp,
            tc.tile_pool(name="yp", bufs=2) as yp,
        ):
            v_sb = main.tile([128, NT, HPC * VSTRIDE], bf16)
            qT_sb = main.tile([128, 4, QF], bf16)
            wpT_sb = main.tile([128, 4, C], bf16)
            outTb = main.tile([128, 4, QF], bf16)
            pTs = [main.tile([128, NT, 2, QF], bf16, name=f"pT{j}")
                   for j in range(NPAIR)]
            nc.vector.memset(outTb[:, :, LQ:QF], 0.0)
            for pT in pTs:
                nc.vector.memset(pT[:, :, :, LQ:QF], 0.0)

            # ones column of V_aug (softmax denominators); t=13 rows 65.. are
            # x-padding -> keep their ones at 0.
            vre = v_sb.rearrange("p t (h e) -> p t h e", e=VSTRIDE)
            ones_f = main.tile([128, NT, HPC, 2], bf16)
            nc.vector.memset(ones_f[:, :, :, 1], 0.0)
            nc.vector.memset(ones_f[:, 0:13, :, 0], 1.0)
            nc.vector.memset(ones_f[64:128, 13, :, 0], 0.0)
            nc.vector.memset(ones_f[64:65, 13, :, 0], 1.0)
            nc.vector.memset(ones_f[0:64, 13, :, 0], 1.0)
            nc.vector.tensor_copy(vre[:, :, :, 64:66], ones_f)

            with (
                tc.tile_pool(name="wk", bufs=1) as wk,
                tc.tile_pool(name="xs", bufs=2) as xs,
                tc.tile_pool(name="kts", bufs=2) as kts,
            ):
                wkT_sb = wk.tile([128, 8, 512], bf16)
                wvT_sb = wk.tile([128, 8, 512], bf16)
                kTbs = {}

                def kv_block(bi, psmm):
                    n0, w = bi * 256, 256
                    if bi == 0:
                        xblk = xblk0
                    else:
                        xblk = xs.tile([128, 8, 256], bf16, tag="xblk")
                        nc.sync.dma_start(xblk, xTr[:, :, n0:n0 + w])
                    kTb = kts.tile([128, 4, 256], bf16, tag="kTb")
                    kTbs[bi] = kTb
                    for mt in range(4):
                        ps = psmm.tile([128, 512], f32, tag="ps")
                        for kj in range(8):
                            nc.tensor.matmul(
                                ps[:, 0:w],
                                wkT_sb[:, kj, mt * 128:(mt + 1) * 128],
                                xblk[:, kj, 0:w],
                                start=(kj == 0), stop=(kj == 7),
                            )
                        nc.vector.tensor_copy(kTb[:, mt, 0:w], ps[:, 0:w])
                    if bi == 0:
                        # needed much later (projection)
                        nc.scalar.dma_start(
                            wpT_sb, wpT.rearrange("(j p) n -> p j n", p=128))
                    for ti in range(2):
                        t = bi * 2 + ti
                        rel = t * 128 - n0
                        ps = psmm.tile([128, 512], f32, tag="ps")
                        for kj in range(8):
                            nc.tensor.matmul(
                                ps,
                                xblk[:, kj, rel:rel + 128],
                                wvT_sb[:, kj, :],
                                start=(kj == 0), stop=(kj == 7),
                            )
                        nc.vector.tensor_copy(
                            vre[:, t, :, 0:64],
                            ps.rearrange("p (h e) -> p h e", e=64),
                        )

                def qk_block(bi, j, stpool, tagA, tagB):
                    """QK -> exp -> *exp(bias) for pair j, block bi's t's."""
                    kTb = kTbs[bi]
                    pT = pTs[j]
                    for ti in range(2):
                        t = bi * 2 + ti
                        bt = biasp.tile([128, 2, LQ], bf16, tag="bt")
                        nc.sync.dma_start(bt, biasx.ap()[j, t])
                        stA_ = stpool.tile([128, C], f32, tag=tagA)
                        stB_ = stpool.tile([128, C], f32, tag=tagB)
                        stA, stB = stA_[:, 0:QF], stB_[:, 0:QF]
                        ks = kTb[:, j, ti * 128:(ti + 1) * 128]
                        for (q0, q1) in QCHUNKS:
                            # adjacent row-tiled pair: concurrent on HW
                            nc.tensor.matmul(
                                stA[:, q0:q1], ks[0:64],
                                qT_sb[0:64, j, q0:q1],
                                start=True, stop=True,
                            )
                            nc.tensor.matmul(
                                stB[:, q0:q1], ks[64:128],
                                qT_sb[64:128, j, q0:q1],
                                start=True, stop=True,
                            )
                        for hsel, st in ((0, stA), (1, stB)):
                            nc.scalar.activation(
                                pT[:, t, hsel, 0:LQ], st[:, 0:LQ],
                                mybir.ActivationFunctionType.Exp,
                            )
                        nc.vector.tensor_mul(
                            out=pT[:, t, :, 0:LQ],
                            in0=pT[:, t, :, 0:LQ], in1=bt,
                        )

                def av_pair(j, psout):
                    """AV (head A fully, then head B) + normalize."""
                    pT = pTs[j]
                    for hsel in (0, 1):
                        ops = psout.tile([66, QF], f32,
                                         tag=("opsA", "opsB")[hsel])
                        h = 2 * j + hsel
                        for t in range(NT):
                            lv = v_sb[:, t, h * VSTRIDE:(h + 1) * VSTRIDE]
                            for (q0, q1) in QCHUNKS:
                                nc.tensor.matmul(
                                    ops[:, q0:q1], lv, pT[:, t, hsel, q0:q1],
                                    start=(t == 0), stop=(t == NT - 1),
                                )
                        rec = recp.tile([1, LQ], f32, tag="rec")
                        nc.vector.reciprocal(rec, ops[64:65, 0:LQ])
                        rbc = recp.tile([64, LQ], f32, tag="rbc")
                        nc.gpsimd.partition_broadcast(rbc, rec)
                        nc.vector.tensor_mul(
                            out=outTb[hsel * 64:(hsel + 1) * 64, j, 0:LQ],
                            in0=ops[0:64, 0:LQ], in1=rbc,
                        )

                def proj_pair(ps0, ps1, jj):
                    for mi, ps in ((0, ps0), (1, ps1)):
                        for (c0, c1) in ((0, 512), (512, C)):
                            nc.tensor.matmul(
                                ps[:, c0:c1],
                                outTb[:, jj, mi * 128:(mi + 1) * 128],
                                wpT_sb[:, jj, c0:c1],
                                start=(jj == 0), stop=(jj == 3),
                            )

                # ---- blocks 0-5 (+ block 6 kv) under the kv PSUM pool ----
                with (
                    tc.tile_pool(name="psmm", bufs=3, space="PSUM") as psmm,
                    tc.tile_pool(name="psst", bufs=1, space="PSUM") as psst,
                ):
                    # first x block + wkT first, split small: DMA transfers
                    # are the startup critical path
                    xblk0 = xs.tile([128, 8, 256], bf16, tag="xblk")
                    wkTr = wkT.rearrange("(j p) m -> p j m", p=128)
                    nc.sync.dma_start(xblk0[:, 0:4], xTr[:, 0:4, 0:256])
                    nc.scalar.dma_start(wkT_sb[:, 0:4], wkTr[:, 0:4])
                    nc.sync.dma_start(xblk0[:, 4:8], xTr[:, 4:8, 0:256])
                    nc.scalar.dma_start(wkT_sb[:, 4:8], wkTr[:, 4:8])
                    nc.gpsimd.dma_start(qT_sb, qT.ap())
                    nc.scalar.dma_start(
                        wvT_sb, wvT.rearrange("(j p) m -> p j m", p=128))
                    for bi in range(6):
                        kv_block(bi, psmm)
                        for j in range(NPAIR):
                            qk_block(bi, j, psst, "stA", "stB")
                    kv_block(6, psmm)

                # ---- last block's QK + AV + normalize + proj mt0/mt1 ----
                # (kv pool closed: the QK slots + AV accumulators fill all 8
                # banks; AV sweeps interleave with the block-6 exp tail)
                with tc.tile_pool(name="psout", bufs=1, space="PSUM") as psout:
                    qk_block(6, 0, psout, "stA6", "stB6")
                    qk_block(6, 1, psout, "stA6", "stB6")
                    av_pair(0, psout)
                    qk_block(6, 2, psout, "stA6", "stB6")
                    av_pair(1, psout)
                    qk_block(6, 3, psout, "stA6", "stB6")
                    av_pair(2, psout)
                    # proj row-tiles mt0/mt1 reuse the QK slots (same size)
                    pp0 = psout.tile([128, C], f32, tag="stA6")
                    pp1 = psout.tile([128, C], f32, tag="stB6")
                    for jj in range(3):
                        proj_pair(pp0, pp1, jj)
                    av_pair(3, psout)
                    proj_pair(pp0, pp1, 3)
                    for mi, ps in ((0, pp0), (1, pp1)):
                        yt = yp.tile([128, C], bf16, tag="yt")
                        if mi == 0:
                            nc.scalar.copy(yt, ps)
                        else:
                            nc.vector.tensor_copy(yt, ps)
                        nc.sync.dma_start(y.ap()[mi], yt)

            # ---- remaining projection row-tiles (wk/xs/kts closed) ----
            with (
                tc.tile_pool(name="pspj2", bufs=2, space="PSUM") as pspj2,
                tc.tile_pool(name="yp2", bufs=3) as yp2,
            ):
                for mt in range(2, 5):
                    m0 = mt * 128
                    mcols = 66 if mt == 4 else 128   # lhsT free width (even)
                    mrows = 65 if mt == 4 else 128   # valid output rows
                    ps = pspj2.tile([128, C], f32, tag="pp")
                    for jj in range(4):
                        for (c0, c1) in ((0, 512), (512, C)):
                            nc.tensor.matmul(
                                ps[:mcols, c0:c1],
                                outTb[:, jj, m0:m0 + mcols],
                                wpT_sb[:, jj, c0:c1],
                                start=(jj == 0), stop=(jj == 3),
                            )
                    yt = yp2.tile([128, C], bf16, tag="yt")
                    if mt == 3:
                        nc.vector.tensor_copy(yt[:mrows], ps[:mrows])
                    else:
                        nc.scalar.copy(yt[:mrows], ps[:mrows])
                    nc.sync.dma_start(y.ap()[mt, 0:mrows], yt[:mrows])

    nc.finalize()
    return nc


_NC_CACHE = None


def _get_nc():
    global _NC_CACHE
    if _NC_CACHE is None:
        _NC_CACHE = _build_nc()
    return _NC_CACHE


def _host_prep(x, q_learned, pos_embed, Wk, Wv, Wp, rpe_W, rp_bucket):
    """Build the 8 per-core input maps."""
    bf = ml_dtypes.bfloat16
    x = np.asarray(x, dtype=np.float32)
    q_ = (np.asarray(q_learned, np.float32) + np.asarray(pos_embed, np.float32))[0]
    Wk = np.asarray(Wk, np.float32)
    Wv = np.asarray(Wv, np.float32)
    Wp = np.asarray(Wp, np.float32)
    rpe_W = np.asarray(rpe_W, np.float32)
    rp_bucket = np.asarray(rp_bucket)

    scale = HD ** -0.5

    # RPE bias, expanded to key-tile layout: [H, n, q] -> pair-major
    qh = q_.reshape(LQ, H, HD)
    rpe_tab = np.einsum('qhd,dn->hqn', qh, rpe_W)                  # (H, LQ, nb)
    rpe = np.take_along_axis(
        rpe_tab, np.broadcast_to(rp_bucket[None], (H, LQ, LQ)), axis=-1
    )                                                              # (H, q, j')
    n_idx = np.arange(NPAD)
    jcol = np.where(n_idx == 0, 0, 1 + (n_idx - 1) % P_SP)         # (NPAD,)
    biasx = rpe[:, :, jcol]                                        # (H, q, n)
    biasx[:, :, NKV:] = 0.0
    biasx = np.exp(biasx)                # multiplicative bias: exp(S)*exp(b)
    biasx = np.ascontiguousarray(biasx.transpose(0, 2, 1))         # (H, n, q)
    # (H=8 per group, n, q) -> [pair, t, p, hsel, q]
    biasg = biasx.reshape(2, NPAIR, 2, NT, 128, LQ).transpose(0, 1, 3, 4, 2, 5)

    # qT per group, scaled, padded: (2, 128, 4, QF)
    qTg = np.zeros((2, 512, QF), np.float32)
    qTg[:, :, :LQ] = (q_.T * scale).reshape(2, 512, LQ)
    qTg = qTg.reshape(2, 4, 128, QF).transpose(0, 2, 1, 3)

    per_group = []
    for g in range(2):
        sl = slice(g * 512, (g + 1) * 512)
        per_group.append({
            "wkT": np.ascontiguousarray(Wk[sl, :].T).astype(bf),
            "wvT": np.ascontiguousarray(Wv[sl, :].T).astype(bf),
            "wpT": np.ascontiguousarray(Wp[:, sl].T).astype(bf),
            "qT": np.ascontiguousarray(qTg[g]).astype(bf),
            "biasx": np.ascontiguousarray(biasg[g]).astype(bf),
        })

    in_maps = []
    for b in range(B):
        xTb = np.zeros((C, NPAD), bf)
        xTb[:, :NKV] = x[b].T.astype(bf)
        for g in range(2):
            m = dict(per_group[g])
            m["xT"] = xTb
            in_maps.append(m)
    return in_maps


def kernel(x, q_learned, pos_embed, Wk, Wv, Wp, bp, rpe_W, rp_bucket):
    in_maps = _host_prep(x, q_learned, pos_embed, Wk, Wv, Wp, rpe_W, rp_bucket)
    nc = _get_nc()

    last_err = None
    for _attempt in range(3):
        try:
            res = run_bass_kernel_spmd(nc, in_maps, core_ids=list(range(NCORES)))
            break
        except Exception as e:  # wedged-device recovery: retry
            last_err = e
    else:
        raise last_err

    bp = np.asarray(bp, np.float32)
    out = np.empty((B, LQ, C), np.float32)
    for b in range(B):
        y0 = res.results[2 * b]["y"].reshape(640, C)[:LQ].astype(np.float32)
        y1 = res.results[2 * b + 1]["y"].reshape(640, C)[:LQ].astype(np.float32)
        out[b] = y0 + y1 + bp
    return out


# revision 3
# speedup vs baseline: 54230.3846x; 1.0241x over previous
"""Trainium2 Bass kernel for nn_CrossRPEAttentionMulti — v8.

Sharding: 8 cores = batch(4) x head-group(2). Per core: one batch, 8 heads.

All matmul operands bf16 (fp32 PSUM accumulation). Bias is applied
multiplicatively after exp (exp(S+b) = exp(S)*exp(b), exp(b) host-side).

Merged pipeline: per x^T block: kT(block) -> QK for all 4 head pairs
(row-tiled head-pair matmuls, adjacent) -> exp on ACT straight from PSUM
-> bf16 multiply by exp(bias) on DVE -> V(block). kv of the next block
overlaps the exps. The LAST block's QK runs after the kv PSUM pool is
closed, so the AV sweeps interleave with its exp tail (PE stays dense).
AV per head (ones column gives denominators), then reciprocal +
partition-broadcast + normalize overlap the next AV sweep. Projection
row-tiles mt0/mt1 accumulate per pair inside the AV loop reusing the QK
PSUM slots; mt2-4 run at the end. Host sums the group partials + bias.
"""
import numpy as np
import ml_dtypes

import concourse.mybir as mybir
import concourse.tile as tile
from concourse import bacc
from concourse.bass_utils import run_bass_kernel_spmd

f32 = mybir.dt.float32
bf16 = mybir.dt.bfloat16

# -- static problem configuration (matches the reference module) --
B, C, H, G = 4, 1024, 16, 24
P_SP = G * G            # 576 spatial patches / modality
LQ = P_SP + 1           # 577 queries
NKV = 3 * P_SP + 1      # 1729 keys/values
HD = C // H             # 64
HPC = 8                 # heads per core (16 heads / 2 groups)
NPAIR = 4               # head pairs per core
NCORES = 8

NPAD = 1792             # keys padded to 14*128
NT = NPAD // 128        # 14 key tiles
QF = LQ + 1             # query free width 578 (even pad)
NBLK = 7                # x^T blocks of 256
VSTRIDE = 66            # per-head V cols: 64 dims + ones col + pad
QCHUNKS = ((0, 512), (512, QF))


def _build_nc():
    nc = bacc.Bacc("TRN2", target_bir_lowering=False, debug=False)

    xT = nc.dram_tensor("xT", [C, NPAD], bf16, kind="ExternalInput")
    wkT = nc.dram_tensor("wkT", [C, 512], bf16, kind="ExternalInput")
    wvT = nc.dram_tensor("wvT", [C, 512], bf16, kind="ExternalInput")
    wpT = nc.dram_tensor("wpT", [512, C], bf16, kind="ExternalInput")
    qT = nc.dram_tensor("qT", [128, 4, QF], bf16, kind="ExternalInput")
    # exp(bias), pair-major: [pair, t, partition, head-in-pair, query]
    biasx = nc.dram_tensor("biasx", [NPAIR, NT, 128, 2, LQ], bf16,
                           kind="ExternalInput")
    y = nc.dram_tensor("y", [5, 128, C], bf16, kind="ExternalOutput")

    xTr = xT.rearrange("(j p) n -> p j n", p=128)

    with tile.TileContext(nc) as tc:
        with (
            tc.tile_pool(name="main", bufs=1) as main,
            tc.tile_pool(name="biasp", bufs=3) as biasp,
            tc.tile_pool(name="recp", bufs=2) as recp,
            tc.tile_pool(name="yp", bufs=2) as yp,
        ):
            v_sb = main.tile([128, NT, HPC * VSTRIDE], bf16)
            qT_sb = main.tile([128, 4, QF], bf16)
            wpT_sb = main.tile([128, 4, C], bf16)
            outTb = main.tile([128, 4, QF], bf16)
            pTs = [main.tile([128, NT, 2, QF], bf16, name=f"pT{j}")
                   for j in range(NPAIR)]
            nc.vector.memset(outTb[:, :, LQ:QF], 0.0)
            for pT in pTs:
                nc.vector.memset(pT[:, :, :, LQ:QF], 0.0)

            # ones column of V_aug (softmax denominators); t=13 rows 65.. are
            # x-padding -> keep their ones at 0.
            vre = v_sb.rearrange("p t (h e) -> p t h e", e=VSTRIDE)
            ones_f = main.tile([128, NT, HPC, 2], bf16)
            nc.vector.memset(ones_f[:, :, :, 1], 0.0)
            nc.vector.memset(ones_f[:, 0:13, :, 0], 1.0)
            nc.vector.memset(ones_f[64:128, 13, :, 0], 0.0)
            nc.vector.memset(ones_f[64:65, 13, :, 0], 1.0)
            nc.vector.memset(ones_f[0:64, 13, :, 0], 1.0)
            nc.vector.tensor_copy(vre[:, :, :, 64:66], ones_f)

            with (
                tc.tile_pool(name="wk", bufs=1) as wk,
                tc.tile_pool(name="xs", bufs=2) as xs,
                tc.tile_pool(name="kts", bufs=2) as kts,
            ):
                wkT_sb = wk.tile([128, 8, 512], bf16)
                wvT_sb = wk.tile([128, 8, 512], bf16)
                kTbs = {}

                def kv_block(bi, psmm):
                    n0, w = bi * 256, 256
                    if bi == 0:
                        xblk = xblk0
                    else:
                        xblk = xs.tile([128, 8, 256], bf16, tag="xblk")
                        nc.sync.dma_start(xblk, xTr[:, :, n0:n0 + w])
                    kTb = kts.tile([128, 4, 256], bf16, tag="kTb")
                    kTbs[bi] = kTb
                    for mt in range(4):
                        ps = psmm.tile([128, 512], f32, tag="ps")
                        for kj in range(8):
                            nc.tensor.matmul(
                                ps[:, 0:w],
                                wkT_sb[:, kj, mt * 128:(mt + 1) * 128],
                                xblk[:, kj, 0:w],
                                start=(kj == 0), stop=(kj == 7),
                            )
                        nc.vector.tensor_copy(kTb[:, mt, 0:w], ps[:, 0:w])
                    if bi == 0:
                        # needed much later (projection)
                        nc.scalar.dma_start(
                            wpT_sb, wpT.rearrange("(j p) n -> p j n", p=128))
                    for ti in range(2):
                        t = bi * 2 + ti
                        rel = t * 128 - n0
                        ps = psmm.tile([128, 512], f32, tag="ps")
                        for kj in range(8):
                            nc.tensor.matmul(
                                ps,
                                xblk[:, kj, rel:rel + 128],
                                wvT_sb[:, kj, :],
                                start=(kj == 0), stop=(kj == 7),
                            )
                        nc.vector.tensor_copy(
                            vre[:, t, :, 0:64],
                            ps.rearrange("p (h e) -> p h e", e=64),
                        )

                def qk_block(bi, j, stpool, tagA, tagB):
                    """QK -> exp -> *exp(bias) for pair j, block bi's t's."""
                    kTb = kTbs[bi]
                    pT = pTs[j]
                    for ti in range(2):
                        t = bi * 2 + ti
                        bt = biasp.tile([128, 2, LQ], bf16, tag="bt")
                        nc.sync.dma_start(bt, biasx.ap()[j, t])
                        stA_ = stpool.tile([128, C], f32, tag=tagA)
                        stB_ = stpool.tile([128, C], f32, tag=tagB)
                        stA, stB = stA_[:, 0:QF], stB_[:, 0:QF]
                        ks = kTb[:, j, ti * 128:(ti + 1) * 128]
                        for (q0, q1) in QCHUNKS:
                            # adjacent row-tiled pair: concurrent on HW
                            nc.tensor.matmul(
                                stA[:, q0:q1], ks[0:64],
                                qT_sb[0:64, j, q0:q1],
                                start=True, stop=True,
                            )
                            nc.tensor.matmul(
                                stB[:, q0:q1], ks[64:128],
                                qT_sb[64:128, j, q0:q1],
                                start=True, stop=True,
                            )
                        for hsel, st in ((0, stA), (1, stB)):
                            nc.scalar.activation(
                                pT[:, t, hsel, 0:LQ], st[:, 0:LQ],
                                mybir.ActivationFunctionType.Exp,
                            )
                        nc.vector.tensor_mul(
                            out=pT[:, t, :, 0:LQ],
                            in0=pT[:, t, :, 0:LQ], in1=bt,
                        )

                def av_pair(j, psout):
                    """AV (head A fully, then head B) + normalize."""
                    pT = pTs[j]
                    for hsel in (0, 1):
                        ops = psout.tile([66, QF], f32,
                                         tag=("opsA", "opsB")[hsel])
                        h = 2 * j + hsel
                        for t in range(NT):
                            lv = v_sb[:, t, h * VSTRIDE:(h + 1) * VSTRIDE]
                            for (q0, q1) in QCHUNKS:
                                nc.tensor.matmul(
                                    ops[:, q0:q1], lv, pT[:, t, hsel, q0:q1],
                                    start=(t == 0), stop=(t == NT - 1),
                                )
                        rec = recp.tile([1, LQ], f32, tag="rec")
                        nc.vector.reciprocal(rec, ops[64:65, 0:LQ])
                        rbc = recp.tile([64, LQ], f32, tag="rbc")
                        nc.gpsimd.partition_broadcast(rbc, rec)
                        nc.vector.tensor_mul(
                            out=outTb[hsel * 64:(hsel + 1) * 64, j, 0:LQ],
                            in0=ops[0:64, 0:LQ], in1=rbc,
                        )

                def proj_pair(ps0, ps1, jj):
                    for mi, ps in ((0, ps0), (1, ps1)):
                        for (c0, c1) in ((0, 512), (512, C)):
                            nc.tensor.matmul(
                                ps[:, c0:c1],
                                outTb[:, jj, mi * 128:(mi + 1) * 128],
                                wpT_sb[:, jj, c0:c1],
                                start=(jj == 0), stop=(jj == 3),
                            )

                # ---- blocks 0-5 (+ block 6 kv) under the kv PSUM pool ----
                with (
                    tc.tile_pool(name="psmm", bufs=3, space="PSUM") as psmm,
                    tc.tile_pool(name="psst", bufs=1, space="PSUM") as psst,
                ):
                    # first x block + wkT first, split small: DMA transfers
                    # are the startup critical path
                    xblk0 = xs.tile([128, 8, 256], bf16, tag="xblk")
                    wkTr = wkT.rearrange("(j p) m -> p j m", p=128)
                    nc.sync.dma_start(xblk0[:, 0:4], xTr[:, 0:4, 0:256])
                    nc.scalar.dma_start(wkT_sb[:, 0:4], wkTr[:, 0:4])
                    nc.sync.dma_start(xblk0[:, 4:8], xTr[:, 4:8, 0:256])
                    nc.scalar.dma_start(wkT_sb[:, 4:8], wkTr[:, 4:8])
                    nc.gpsimd.dma_start(qT_sb, qT.ap())
                    nc.scalar.dma_start(
                        wvT_sb, wvT.rearrange("(j p) m -> p j m", p=128))
                    for bi in range(6):
                        kv_block(bi, psmm)
                        for j in range(NPAIR):
                            qk_block(bi, j, psst, "stA", "stB")
                    kv_block(6, psmm)

                # ---- last block's QK + AV + normalize + proj mt0/mt1 ----
                # (kv pool closed: the QK slots + AV accumulators fill all 8
                # banks; AV sweeps interleave with the block-6 exp tail)
                with tc.tile_pool(name="psout", bufs=1, space="PSUM") as psout:
                    qk_block(6, 0, psout, "stA6", "stB6")
                    qk_block(6, 1, psout, "stA6", "stB6")
                    av_pair(0, psout)
                    qk_block(6, 2, psout, "stA6", "stB6")
                    av_pair(1, psout)
                    qk_block(6, 3, psout, "stA6", "stB6")
                    av_pair(2, psout)
                    av_pair(3, psout)
                    # proj row-tiles mt0/mt1 reuse the QK slots (same size);
                    # jj 0-2 fill the last normalization chain's PE gap
                    pp0 = psout.tile([128, C], f32, tag="stA6")
                    pp1 = psout.tile([128, C], f32, tag="stB6")
                    for jj in range(4):
                        proj_pair(pp0, pp1, jj)
                    for mi, ps in ((0, pp0), (1, pp1)):
                        yt = yp.tile([128, C], bf16, tag="yt")
                        if mi == 0:
                            nc.scalar.copy(yt, ps)
                        else:
                            nc.vector.tensor_copy(yt, ps)
                        nc.sync.dma_start(y.ap()[mi], yt)

            # ---- remaining projection row-tiles (wk/xs/kts closed) ----
            with (
                tc.tile_pool(name="pspj2", bufs=2, space="PSUM") as pspj2,
                tc.tile_pool(name="yp2", bufs=3) as yp2,
            ):
                for mt in range(2, 5):
                    m0 = mt * 128
                    mcols = 66 if mt == 4 else 128   # lhsT free width (even)
                    mrows = 65 if mt == 4 else 128   # valid output rows
                    ps = pspj2.tile([128, C], f32, tag="pp")
                    for jj in range(4):
                        for (c0, c1) in ((0, 512), (512, C)):
                            nc.tensor.matmul(
                                ps[:mcols, c0:c1],
                                outTb[:, jj, m0:m0 + mcols],
                                wpT_sb[:, jj, c0:c1],
                                start=(jj == 0), stop=(jj == 3),
                            )
                    yt = yp2.tile([128, C], bf16, tag="yt")
                    if mt == 3:
                        nc.vector.tensor_copy(yt[:mrows], ps[:mrows])
                    else:
                        nc.scalar.copy(yt[:mrows], ps[:mrows])
                    nc.sync.dma_start(y.ap()[mt, 0:mrows], yt[:mrows])

    nc.finalize()
    return nc


_NC_CACHE = None


def _get_nc():
    global _NC_CACHE
    if _NC_CACHE is None:
        _NC_CACHE = _build_nc()
    return _NC_CACHE


def _host_prep(x, q_learned, pos_embed, Wk, Wv, Wp, rpe_W, rp_bucket):
    """Build the 8 per-core input maps."""
    bf = ml_dtypes.bfloat16
    x = np.asarray(x, dtype=np.float32)
    q_ = (np.asarray(q_learned, np.float32) + np.asarray(pos_embed, np.float32))[0]
    Wk = np.asarray(Wk, np.float32)
    Wv = np.asarray(Wv, np.float32)
    Wp = np.asarray(Wp, np.float32)
    rpe_W = np.asarray(rpe_W, np.float32)
    rp_bucket = np.asarray(rp_bucket)

    scale = HD ** -0.5

    # RPE bias, expanded to key-tile layout: [H, n, q] -> pair-major
    qh = q_.reshape(LQ, H, HD)
    rpe_tab = np.einsum('qhd,dn->hqn', qh, rpe_W)                  # (H, LQ, nb)
    rpe = np.take_along_axis(
        rpe_tab, np.broadcast_to(rp_bucket[None], (H, LQ, LQ)), axis=-1
    )                                                              # (H, q, j')
    n_idx = np.arange(NPAD)
    jcol = np.where(n_idx == 0, 0, 1 + (n_idx - 1) % P_SP)         # (NPAD,)
    biasx = rpe[:, :, jcol]                                        # (H, q, n)
    biasx[:, :, NKV:] = 0.0
    biasx = np.exp(biasx)                # multiplicative bias: exp(S)*exp(b)
    biasx = np.ascontiguousarray(biasx.transpose(0, 2, 1))         # (H, n, q)
    # (H=8 per group, n, q) -> [pair, t, p, hsel, q]
    biasg = biasx.reshape(2, NPAIR, 2, NT, 128, LQ).transpose(0, 1, 3, 4, 2, 5)

    # qT per group, scaled, padded: (2, 128, 4, QF)
    qTg = np.zeros((2, 512, QF), np.float32)
    qTg[:, :, :LQ] = (q_.T * scale).reshape(2, 512, LQ)
    qTg = qTg.reshape(2, 4, 128, QF).transpose(0, 2, 1, 3)

    per_group = []
    for g in range(2):
        sl = slice(g * 512, (g + 1) * 512)
        per_group.append({
            "wkT": np.ascontiguousarray(Wk[sl, :].T).astype(bf),
            "wvT": np.ascontiguousarray(Wv[sl, :].T).astype(bf),
            "wpT": np.ascontiguousarray(Wp[:, sl].T).astype(bf),
            "qT": np.ascontiguousarray(qTg[g]).astype(bf),
            "biasx": np.ascontiguousarray(biasg[g]).astype(bf),
        })

    in_maps = []
    for b in range(B):
        xTb = np.zeros((C, NPAD), bf)
        xTb[:, :NKV] = x[b].T.astype(bf)
        for g in range(2):
            m = dict(per_group[g])
            m["xT"] = xTb
            in_maps.append(m)
    return in_maps


def kernel(x, q_learned, pos_embed, Wk, Wv, Wp, bp, rpe_W, rp_bucket):
    in_maps = _host_prep(x, q_learned, pos_embed, Wk, Wv, Wp, rpe_W, rp_bucket)
    nc = _get_nc()

    last_err = None
    for _attempt in range(3):
        try:
            res = run_bass_kernel_spmd(nc, in_maps, core_ids=list(range(NCORES)))
            break
        except Exception as e:  # wedged-device recovery: retry
            last_err = e
    else:
        raise last_err

    bp = np.asarray(bp, np.float32)
    out = np.empty((B, LQ, C), np.float32)
    for b in range(B):
        y0 = res.results[2 * b]["y"].reshape(640, C)[:LQ].astype(np.float32)
        y1 = res.results[2 * b + 1]["y"].reshape(640, C)[:LQ].astype(np.float32)
        out[b] = y0 + y1 + bp
    return out


# revision 4
# speedup vs baseline: 54404.4169x; 1.0032x over previous
"""Trainium2 Bass kernel for nn_CrossRPEAttentionMulti — v8.

Sharding: 8 cores = batch(4) x head-group(2). Per core: one batch, 8 heads.

All matmul operands bf16 (fp32 PSUM accumulation). Bias is applied
multiplicatively after exp (exp(S+b) = exp(S)*exp(b), exp(b) host-side).

Merged pipeline: per x^T block: kT(block) -> QK for all 4 head pairs
(row-tiled head-pair matmuls, adjacent) -> exp on ACT straight from PSUM
-> bf16 multiply by exp(bias) on DVE -> V(block). kv of the next block
overlaps the exps. The LAST block's QK runs after the kv PSUM pool is
closed, so the AV sweeps interleave with its exp tail (PE stays dense).
AV per head (ones column gives denominators), then reciprocal +
partition-broadcast + normalize overlap the next AV sweep. Projection
row-tiles mt0/mt1 accumulate per pair inside the AV loop reusing the QK
PSUM slots; mt2-4 run at the end. Host sums the group partials + bias.
"""
import numpy as np
import ml_dtypes

import concourse.mybir as mybir
import concourse.tile as tile
from concourse import bacc
from concourse.bass_utils import run_bass_kernel_spmd

f32 = mybir.dt.float32
bf16 = mybir.dt.bfloat16

# -- static problem configuration (matches the reference module) --
B, C, H, G = 4, 1024, 16, 24
P_SP = G * G            # 576 spatial patches / modality
LQ = P_SP + 1           # 577 queries
NKV = 3 * P_SP + 1      # 1729 keys/values
HD = C // H             # 64
HPC = 8                 # heads per core (16 heads / 2 groups)
NPAIR = 4               # head pairs per core
NCORES = 8

NPAD = 1792             # keys padded to 14*128
NT = NPAD // 128        # 14 key tiles
QF = LQ + 1             # query free width 578 (even pad)
NBLK = 7                # x^T blocks of 256
VSTRIDE = 66            # per-head V cols: 64 dims + ones col + pad
QCHUNKS = ((0, 512), (512, QF))


def _build_nc():
    nc = bacc.Bacc("TRN2", target_bir_lowering=False, debug=False)

    xT = nc.dram_tensor("xT", [C, NPAD], bf16, kind="ExternalInput")
    wkT = nc.dram_tensor("wkT", [C, 512], bf16, kind="ExternalInput")
    wvT = nc.dram_tensor("wvT", [C, 512], bf16, kind="ExternalInput")
    wpT = nc.dram_tensor("wpT", [512, C], bf16, kind="ExternalInput")
    qT = nc.dram_tensor("qT", [128, 4, QF], bf16, kind="ExternalInput")
    # exp(bias), pair-major: [pair, t, partition, head-in-pair, query]
    biasx = nc.dram_tensor("biasx", [NPAIR, NT, 128, 2, LQ], bf16,
                           kind="ExternalInput")
    y = nc.dram_tensor("y", [5, 128, C], bf16, kind="ExternalOutput")

    xTr = xT.rearrange("(j p) n -> p j n", p=128)

    with tile.TileContext(nc) as tc:
        with (
            tc.tile_pool(name="main", bufs=1) as main,
            tc.tile_pool(name="biasp", bufs=3) as biasp,
            tc.tile_pool(name="recp", bufs=2) as recp,
            tc.tile_pool(name="yp", bufs=2) as yp,
        ):
            v_sb = main.tile([128, NT, HPC * VSTRIDE], bf16)
            qT_sb = main.tile([128, 4, QF], bf16)
            wpT_sb = main.tile([128, 4, C], bf16)
            outTb = main.tile([128, 4, QF], bf16)
            pTs = [main.tile([128, NT, 2, QF], bf16, name=f"pT{j}")
                   for j in range(NPAIR)]
            nc.vector.memset(outTb[:, :, LQ:QF], 0.0)
            for pT in pTs:
                nc.vector.memset(pT[:, :, :, LQ:QF], 0.0)

            # ones column of V_aug (softmax denominators); t=13 rows 65.. are
            # x-padding -> keep their ones at 0.
            vre = v_sb.rearrange("p t (h e) -> p t h e", e=VSTRIDE)
            ones_f = main.tile([128, NT, HPC, 2], bf16)
            nc.vector.memset(ones_f[:, :, :, 1], 0.0)
            nc.vector.memset(ones_f[:, 0:13, :, 0], 1.0)
            nc.vector.memset(ones_f[64:128, 13, :, 0], 0.0)
            nc.vector.memset(ones_f[64:65, 13, :, 0], 1.0)
            nc.vector.memset(ones_f[0:64, 13, :, 0], 1.0)
            nc.vector.tensor_copy(vre[:, :, :, 64:66], ones_f)

            with (
                tc.tile_pool(name="wk", bufs=1) as wk,
                tc.tile_pool(name="xs", bufs=2) as xs,
                tc.tile_pool(name="kts", bufs=2) as kts,
            ):
                wkT_sb = wk.tile([128, 8, 512], bf16)
                wvT_sb = wk.tile([128, 8, 512], bf16)
                kTbs = {}

                def kv_block(bi, psmm):
                    n0, w = bi * 256, 256
                    if bi == 0:
                        xblk = xblk0
                    else:
                        xblk = xs.tile([128, 8, 256], bf16, tag="xblk")
                        nc.sync.dma_start(xblk, xTr[:, :, n0:n0 + w])
                    kTb = kts.tile([128, 4, 256], bf16, tag="kTb")
                    kTbs[bi] = kTb
                    for mt in range(4):
                        ps = psmm.tile([128, 512], f32, tag="ps")
                        for kj in range(8):
                            nc.tensor.matmul(
                                ps[:, 0:w],
                                wkT_sb[:, kj, mt * 128:(mt + 1) * 128],
                                xblk[:, kj, 0:w],
                                start=(kj == 0), stop=(kj == 7),
                            )
                        nc.vector.tensor_copy(kTb[:, mt, 0:w], ps[:, 0:w])
                    if bi == 0:
                        # needed much later (projection)
                        nc.scalar.dma_start(
                            wpT_sb, wpT.rearrange("(j p) n -> p j n", p=128))
                    for ti in range(2):
                        t = bi * 2 + ti
                        rel = t * 128 - n0
                        ps = psmm.tile([128, 512], f32, tag="ps")
                        for kj in range(8):
                            nc.tensor.matmul(
                                ps,
                                xblk[:, kj, rel:rel + 128],
                                wvT_sb[:, kj, :],
                                start=(kj == 0), stop=(kj == 7),
                            )
                        nc.vector.tensor_copy(
                            vre[:, t, :, 0:64],
                            ps.rearrange("p (h e) -> p h e", e=64),
                        )

                def qk_block(bi, j, stpool, tagA, tagB):
                    """QK -> exp -> *exp(bias) for pair j, block bi's t's."""
                    kTb = kTbs[bi]
                    pT = pTs[j]
                    for ti in range(2):
                        t = bi * 2 + ti
                        bt = biasp.tile([128, 2, LQ], bf16, tag="bt")
                        nc.sync.dma_start(bt, biasx.ap()[j, t])
                        stA_ = stpool.tile([128, C], f32, tag=tagA)
                        stB_ = stpool.tile([128, C], f32, tag=tagB)
                        stA, stB = stA_[:, 0:QF], stB_[:, 0:QF]
                        ks = kTb[:, j, ti * 128:(ti + 1) * 128]
                        for (q0, q1) in QCHUNKS:
                            # adjacent row-tiled pair: concurrent on HW
                            nc.tensor.matmul(
                                stA[:, q0:q1], ks[0:64],
                                qT_sb[0:64, j, q0:q1],
                                start=True, stop=True,
                            )
                            nc.tensor.matmul(
                                stB[:, q0:q1], ks[64:128],
                                qT_sb[64:128, j, q0:q1],
                                start=True, stop=True,
                            )
                        for hsel, st in ((0, stA), (1, stB)):
                            nc.scalar.activation(
                                pT[:, t, hsel, 0:LQ], st[:, 0:LQ],
                                mybir.ActivationFunctionType.Exp,
                            )
                        nc.vector.tensor_mul(
                            out=pT[:, t, :, 0:LQ],
                            in0=pT[:, t, :, 0:LQ], in1=bt,
                        )

                def av_pair(j, psout):
                    """AV (head A fully, then head B) + normalize."""
                    pT = pTs[j]
                    for hsel in (0, 1):
                        ops = psout.tile([66, QF], f32,
                                         tag=("opsA", "opsB")[hsel])
                        h = 2 * j + hsel
                        for t in range(NT):
                            lv = v_sb[:, t, h * VSTRIDE:(h + 1) * VSTRIDE]
                            for (q0, q1) in QCHUNKS:
                                nc.tensor.matmul(
                                    ops[:, q0:q1], lv, pT[:, t, hsel, q0:q1],
                                    start=(t == 0), stop=(t == NT - 1),
                                )
                        rec = recp.tile([1, LQ], f32, tag="rec")
                        nc.vector.reciprocal(rec, ops[64:65, 0:LQ])
                        rbc = recp.tile([64, LQ], f32, tag="rbc")
                        nc.gpsimd.partition_broadcast(rbc, rec)
                        nc.vector.tensor_mul(
                            out=outTb[hsel * 64:(hsel + 1) * 64, j, 0:LQ],
                            in0=ops[0:64, 0:LQ], in1=rbc,
                        )

                def proj_pair(ps0, ps1, jj):
                    for mi, ps in ((0, ps0), (1, ps1)):
                        for (c0, c1) in ((0, 512), (512, C)):
                            nc.tensor.matmul(
                                ps[:, c0:c1],
                                outTb[:, jj, mi * 128:(mi + 1) * 128],
                                wpT_sb[:, jj, c0:c1],
                                start=(jj == 0), stop=(jj == 3),
                            )

                # ---- blocks 0-5 (+ block 6 kv) under the kv PSUM pool ----
                with (
                    tc.tile_pool(name="psmm", bufs=3, space="PSUM") as psmm,
                    tc.tile_pool(name="psst", bufs=1, space="PSUM") as psst,
                ):
                    # PE warm-up: dummy matmuls on already-memset data keep
                    # the PE busy through the startup DMA window so the real
                    # kv matmuls start at the full (warm) clock instead of
                    # paying the cold-ramp penalty. Results are never read.
                    ones_flat = ones_f.rearrange("p a b c -> p (a b c)")
                    for _d in range(14):
                        psd = psmm.tile([128, 512], f32, tag="ps")
                        nc.tensor.matmul(
                            psd[:, 0:224], ones_flat[:, 0:128], ones_flat,
                            start=True, stop=True,
                        )
                    # first x block + wkT first, split small: DMA transfers
                    # are the startup critical path
                    xblk0 = xs.tile([128, 8, 256], bf16, tag="xblk")
                    wkTr = wkT.rearrange("(j p) m -> p j m", p=128)
                    nc.sync.dma_start(xblk0[:, 0:4], xTr[:, 0:4, 0:256])
                    nc.scalar.dma_start(wkT_sb[:, 0:4], wkTr[:, 0:4])
                    nc.sync.dma_start(xblk0[:, 4:8], xTr[:, 4:8, 0:256])
                    nc.scalar.dma_start(wkT_sb[:, 4:8], wkTr[:, 4:8])
                    nc.gpsimd.dma_start(qT_sb, qT.ap())
                    nc.scalar.dma_start(
                        wvT_sb, wvT.rearrange("(j p) m -> p j m", p=128))
                    for bi in range(6):
                        kv_block(bi, psmm)
                        for j in range(NPAIR):
                            qk_block(bi, j, psst, "stA", "stB")
                    kv_block(6, psmm)

                # ---- last block's QK + AV + normalize + proj mt0/mt1 ----
                # (kv pool closed: the QK slots + AV accumulators fill all 8
                # banks; AV sweeps interleave with the block-6 exp tail)
                with tc.tile_pool(name="psout", bufs=1, space="PSUM") as psout:
                    qk_block(6, 0, psout, "stA6", "stB6")
                    qk_block(6, 1, psout, "stA6", "stB6")
                    av_pair(0, psout)
                    qk_block(6, 2, psout, "stA6", "stB6")
                    av_pair(1, psout)
                    qk_block(6, 3, psout, "stA6", "stB6")
                    av_pair(2, psout)
                    av_pair(3, psout)
                    # proj row-tiles mt0/mt1 reuse the QK slots (same size);
                    # jj 0-2 fill the last normalization chain's PE gap
                    pp0 = psout.tile([128, C], f32, tag="stA6")
                    pp1 = psout.tile([128, C], f32, tag="stB6")
                    for jj in range(4):
                        proj_pair(pp0, pp1, jj)
                    for mi, ps in ((0, pp0), (1, pp1)):
                        yt = yp.tile([128, C], bf16, tag="yt")
                        if mi == 0:
                            nc.scalar.copy(yt, ps)
                        else:
                            nc.vector.tensor_copy(yt, ps)
                        nc.sync.dma_start(y.ap()[mi], yt)

            # ---- remaining projection row-tiles (wk/xs/kts closed) ----
            with (
                tc.tile_pool(name="pspj2", bufs=2, space="PSUM") as pspj2,
                tc.tile_pool(name="yp2", bufs=3) as yp2,
            ):
                for mt in range(2, 5):
                    m0 = mt * 128
                    mcols = 66 if mt == 4 else 128   # lhsT free width (even)
                    mrows = 65 if mt == 4 else 128   # valid output rows
                    ps = pspj2.tile([128, C], f32, tag="pp")
                    for jj in range(4):
                        for (c0, c1) in ((0, 512), (512, C)):
                            nc.tensor.matmul(
                                ps[:mcols, c0:c1],
                                outTb[:, jj, m0:m0 + mcols],
                                wpT_sb[:, jj, c0:c1],
                                start=(jj == 0), stop=(jj == 3),
                            )
                    yt = yp2.tile([128, C], bf16, tag="yt")
                    if mt == 3:
                        nc.vector.tensor_copy(yt[:mrows], ps[:mrows])
                    else:
                        nc.scalar.copy(yt[:mrows], ps[:mrows])
                    nc.sync.dma_start(y.ap()[mt, 0:mrows], yt[:mrows])

    nc.finalize()
    return nc


_NC_CACHE = None


def _get_nc():
    global _NC_CACHE
    if _NC_CACHE is None:
        _NC_CACHE = _build_nc()
    return _NC_CACHE


def _host_prep(x, q_learned, pos_embed, Wk, Wv, Wp, rpe_W, rp_bucket):
    """Build the 8 per-core input maps."""
    bf = ml_dtypes.bfloat16
    x = np.asarray(x, dtype=np.float32)
    q_ = (np.asarray(q_learned, np.float32) + np.asarray(pos_embed, np.float32))[0]
    Wk = np.asarray(Wk, np.float32)
    Wv = np.asarray(Wv, np.float32)
    Wp = np.asarray(Wp, np.float32)
    rpe_W = np.asarray(rpe_W, np.float32)
    rp_bucket = np.asarray(rp_bucket)

    scale = HD ** -0.5

    # RPE bias, expanded to key-tile layout: [H, n, q] -> pair-major
    qh = q_.reshape(LQ, H, HD)
    rpe_tab = np.einsum('qhd,dn->hqn', qh, rpe_W)                  # (H, LQ, nb)
    rpe = np.take_along_axis(
        rpe_tab, np.broadcast_to(rp_bucket[None], (H, LQ, LQ)), axis=-1
    )                                                              # (H, q, j')
    n_idx = np.arange(NPAD)
    jcol = np.where(n_idx == 0, 0, 1 + (n_idx - 1) % P_SP)         # (NPAD,)
    biasx = rpe[:, :, jcol]                                        # (H, q, n)
    biasx[:, :, NKV:] = 0.0
    biasx = np.exp(biasx)                # multiplicative bias: exp(S)*exp(b)
    biasx = np.ascontiguousarray(biasx.transpose(0, 2, 1))         # (H, n, q)
    # (H=8 per group, n, q) -> [pair, t, p, hsel, q]
    biasg = biasx.reshape(2, NPAIR, 2, NT, 128, LQ).transpose(0, 1, 3, 4, 2, 5)

    # qT per group, scaled, padded: (2, 128, 4, QF)
    qTg = np.zeros((2, 512, QF), np.float32)
    qTg[:, :, :LQ] = (q_.T * scale).reshape(2, 512, LQ)
    qTg = qTg.reshape(2, 4, 128, QF).transpose(0, 2, 1, 3)

    per_group = []
    for g in range(2):
        sl = slice(g * 512, (g + 1) * 512)
        per_group.append({
            "wkT": np.ascontiguousarray(Wk[sl, :].T).astype(bf),
            "wvT": np.ascontiguousarray(Wv[sl, :].T).astype(bf),
            "wpT": np.ascontiguousarray(Wp[:, sl].T).astype(bf),
            "qT": np.ascontiguousarray(qTg[g]).astype(bf),
            "biasx": np.ascontiguousarray(biasg[g]).astype(bf),
        })

    in_maps = []
    for b in range(B):
        xTb = np.zeros((C, NPAD), bf)
        xTb[:, :NKV] = x[b].T.astype(bf)
        for g in range(2):
            m = dict(per_group[g])
            m["xT"] = xTb
            in_maps.append(m)
    return in_maps


def kernel(x, q_learned, pos_embed, Wk, Wv, Wp, bp, rpe_W, rp_bucket):
    in_maps = _host_prep(x, q_learned, pos_embed, Wk, Wv, Wp, rpe_W, rp_bucket)
    nc = _get_nc()

    last_err = None
    for _attempt in range(3):
        try:
            res = run_bass_kernel_spmd(nc, in_maps, core_ids=list(range(NCORES)))
            break
        except Exception as e:  # wedged-device recovery: retry
            last_err = e
    else:
        raise last_err

    bp = np.asarray(bp, np.float32)
    out = np.empty((B, LQ, C), np.float32)
    for b in range(B):
        y0 = res.results[2 * b]["y"].reshape(640, C)[:LQ].astype(np.float32)
        y1 = res.results[2 * b + 1]["y"].reshape(640, C)[:LQ].astype(np.float32)
        out[b] = y0 + y1 + bp
    return out


# revision 5
# speedup vs baseline: 55237.0016x; 1.0153x over previous
"""Trainium2 Bass kernel for nn_CrossRPEAttentionMulti — v8.

Sharding: 8 cores = batch(4) x head-group(2). Per core: one batch, 8 heads.

All matmul operands bf16 (fp32 PSUM accumulation). Bias is applied
multiplicatively after exp (exp(S+b) = exp(S)*exp(b), exp(b) host-side).

Merged pipeline: per x^T block: kT(block) -> QK for all 4 head pairs
(row-tiled head-pair matmuls, adjacent) -> exp on ACT straight from PSUM
-> bf16 multiply by exp(bias) on DVE -> V(block). kv of the next block
overlaps the exps. The LAST block's QK runs after the kv PSUM pool is
closed, so the AV sweeps interleave with its exp tail (PE stays dense).
AV per head (ones column gives denominators), then reciprocal +
partition-broadcast + normalize overlap the next AV sweep. Projection
row-tiles mt0/mt1 accumulate per pair inside the AV loop reusing the QK
PSUM slots; mt2-4 run at the end. Host sums the group partials + bias.
"""
import numpy as np
import ml_dtypes

import concourse.mybir as mybir
import concourse.tile as tile
from concourse import bacc
from concourse.bass_utils import run_bass_kernel_spmd

f32 = mybir.dt.float32
bf16 = mybir.dt.bfloat16

# -- static problem configuration (matches the reference module) --
B, C, H, G = 4, 1024, 16, 24
P_SP = G * G            # 576 spatial patches / modality
LQ = P_SP + 1           # 577 queries
NKV = 3 * P_SP + 1      # 1729 keys/values
HD = C // H             # 64
HPC = 8                 # heads per core (16 heads / 2 groups)
NPAIR = 4               # head pairs per core
NCORES = 8

NPAD = 1792             # keys padded to 14*128
NT = NPAD // 128        # 14 key tiles
QF = LQ + 1             # query free width 578 (even pad)
NBLK = 7                # x^T blocks of 256
VSTRIDE = 66            # per-head V cols: 64 dims + ones col + pad
QCHUNKS = ((0, 512), (512, QF))


def _build_nc():
    nc = bacc.Bacc("TRN2", target_bir_lowering=False, debug=False)

    xT = nc.dram_tensor("xT", [C, NPAD], bf16, kind="ExternalInput")
    wkT = nc.dram_tensor("wkT", [C, 512], bf16, kind="ExternalInput")
    wvT = nc.dram_tensor("wvT", [C, 512], bf16, kind="ExternalInput")
    wpT = nc.dram_tensor("wpT", [512, C], bf16, kind="ExternalInput")
    qT = nc.dram_tensor("qT", [128, 4, QF], bf16, kind="ExternalInput")
    # exp(bias), pair-major: [pair, t, partition, head-in-pair, query]
    biasx = nc.dram_tensor("biasx", [NPAIR, NT, 128, 2, LQ], bf16,
                           kind="ExternalInput")
    y = nc.dram_tensor("y", [5, 128, C], bf16, kind="ExternalOutput")

    xTr = xT.rearrange("(j p) n -> p j n", p=128)

    with tile.TileContext(nc) as tc:
        with (
            tc.tile_pool(name="main", bufs=1) as main,
            tc.tile_pool(name="biasp", bufs=3) as biasp,
            tc.tile_pool(name="recp", bufs=2) as recp,
            tc.tile_pool(name="yp", bufs=2) as yp,
        ):
            v_sb = main.tile([128, NT, HPC * VSTRIDE], bf16)
            qT_sb = main.tile([128, 4, QF], bf16)
            wpT_sb = main.tile([128, 4, C], bf16)
            outTb = main.tile([128, 4, QF], bf16)
            pTs = [main.tile([128, NT, 2, QF], bf16, name=f"pT{j}")
                   for j in range(NPAIR)]
            nc.vector.memset(outTb[:, :, LQ:QF], 0.0)
            for pT in pTs:
                nc.vector.memset(pT[:, :, :, LQ:QF], 0.0)

            # ones column of V_aug (softmax denominators); t=13 rows 65.. are
            # x-padding -> keep their ones at 0.
            vre = v_sb.rearrange("p t (h e) -> p t h e", e=VSTRIDE)
            ones_f = main.tile([128, NT, HPC, 2], bf16)
            nc.vector.memset(ones_f[:, :, :, 1], 0.0)
            nc.vector.memset(ones_f[:, 0:13, :, 0], 1.0)
            nc.vector.memset(ones_f[64:128, 13, :, 0], 0.0)
            nc.vector.memset(ones_f[64:65, 13, :, 0], 1.0)
            nc.vector.memset(ones_f[0:64, 13, :, 0], 1.0)
            nc.vector.tensor_copy(vre[:, :, :, 64:66], ones_f)

            with (
                tc.tile_pool(name="wk", bufs=1) as wk,
                tc.tile_pool(name="xs", bufs=2) as xs,
                tc.tile_pool(name="kts", bufs=2) as kts,
            ):
                wkT_sb = wk.tile([128, 8, 512], bf16)
                wvT_sb = wk.tile([128, 8, 512], bf16)
                kTbs = {}

                xbs = {}

                def kt_part(bi, psmm):
                    n0, w = bi * 256, 256
                    if bi == 0:
                        xblk = xblk0
                    elif bi == 1:
                        xblk = xblk1
                    else:
                        xblk = xs.tile([128, 8, 256], bf16, tag="xblk")
                        nc.sync.dma_start(xblk, xTr[:, :, n0:n0 + w])
                    xbs[bi] = xblk
                    kTb = kts.tile([128, 4, 256], bf16, tag="kTb")
                    kTbs[bi] = kTb
                    for mt in range(4):
                        ps = psmm.tile([128, 512], f32, tag="ps")
                        for kj in range(8):
                            nc.tensor.matmul(
                                ps[:, 0:w],
                                wkT_sb[:, kj, mt * 128:(mt + 1) * 128],
                                xblk[:, kj, 0:w],
                                start=(kj == 0), stop=(kj == 7),
                            )
                        nc.vector.tensor_copy(kTb[:, mt, 0:w], ps[:, 0:w])
                    if bi == 0:
                        # needed much later (projection)
                        nc.scalar.dma_start(
                            wpT_sb, wpT.rearrange("(j p) n -> p j n", p=128))

                def v_part(bi, psmm):
                    n0 = bi * 256
                    xblk = xbs[bi]
                    for ti in range(2):
                        t = bi * 2 + ti
                        rel = t * 128 - n0
                        ps = psmm.tile([128, 512], f32, tag="ps")
                        for kj in range(8):
                            nc.tensor.matmul(
                                ps,
                                xblk[:, kj, rel:rel + 128],
                                wvT_sb[:, kj, :],
                                start=(kj == 0), stop=(kj == 7),
                            )
                        nc.vector.tensor_copy(
                            vre[:, t, :, 0:64],
                            ps.rearrange("p (h e) -> p h e", e=64),
                        )

                def kv_block(bi, psmm):
                    kt_part(bi, psmm)
                    v_part(bi, psmm)

                def qk_block(bi, j, stpool, tagA, tagB):
                    """QK -> exp -> *exp(bias) for pair j, block bi's t's."""
                    kTb = kTbs[bi]
                    pT = pTs[j]
                    for ti in range(2):
                        t = bi * 2 + ti
                        bt = biasp.tile([128, 2, LQ], bf16, tag="bt")
                        nc.sync.dma_start(bt, biasx.ap()[j, t])
                        stA_ = stpool.tile([128, C], f32, tag=tagA)
                        stB_ = stpool.tile([128, C], f32, tag=tagB)
                        stA, stB = stA_[:, 0:QF], stB_[:, 0:QF]
                        ks = kTb[:, j, ti * 128:(ti + 1) * 128]
                        for (q0, q1) in QCHUNKS:
                            # adjacent row-tiled pair: concurrent on HW
                            nc.tensor.matmul(
                                stA[:, q0:q1], ks[0:64],
                                qT_sb[0:64, j, q0:q1],
                                start=True, stop=True,
                            )
                            nc.tensor.matmul(
                                stB[:, q0:q1], ks[64:128],
                                qT_sb[64:128, j, q0:q1],
                                start=True, stop=True,
                            )
                        for hsel, st in ((0, stA), (1, stB)):
                            nc.scalar.activation(
                                pT[:, t, hsel, 0:LQ], st[:, 0:LQ],
                                mybir.ActivationFunctionType.Exp,
                            )
                        nc.vector.tensor_mul(
                            out=pT[:, t, :, 0:LQ],
                            in0=pT[:, t, :, 0:LQ], in1=bt,
                        )

                def av_pair(j, psout):
                    """AV (head A fully, then head B) + normalize."""
                    pT = pTs[j]
                    for hsel in (0, 1):
                        ops = psout.tile([66, QF], f32,
                                         tag=("opsA", "opsB")[hsel])
                        h = 2 * j + hsel
                        for t in range(NT):
                            lv = v_sb[:, t, h * VSTRIDE:(h + 1) * VSTRIDE]
                            for (q0, q1) in QCHUNKS:
                                nc.tensor.matmul(
                                    ops[:, q0:q1], lv, pT[:, t, hsel, q0:q1],
                                    start=(t == 0), stop=(t == NT - 1),
                                )
                        rec = recp.tile([1, LQ], f32, tag="rec")
                        nc.vector.reciprocal(rec, ops[64:65, 0:LQ])
                        rbc = recp.tile([64, LQ], f32, tag="rbc")
                        nc.gpsimd.partition_broadcast(rbc, rec)
                        nc.vector.tensor_mul(
                            out=outTb[hsel * 64:(hsel + 1) * 64, j, 0:LQ],
                            in0=ops[0:64, 0:LQ], in1=rbc,
                        )

                def proj_pair(ps0, ps1, jj):
                    for mi, ps in ((0, ps0), (1, ps1)):
                        for (c0, c1) in ((0, 512), (512, C)):
                            nc.tensor.matmul(
                                ps[:, c0:c1],
                                outTb[:, jj, mi * 128:(mi + 1) * 128],
                                wpT_sb[:, jj, c0:c1],
                                start=(jj == 0), stop=(jj == 3),
                            )

                # ---- blocks 0-5 (+ block 6 kv) under the kv PSUM pool ----
                with (
                    tc.tile_pool(name="psmm", bufs=3, space="PSUM") as psmm,
                    tc.tile_pool(name="psst", bufs=1, space="PSUM") as psst,
                ):
                    # PE warm-up: dummy matmuls on already-memset data keep
                    # the PE busy through the startup DMA window so the real
                    # kv matmuls start at the full (warm) clock instead of
                    # paying the cold-ramp penalty. Results are never read.
                    ones_flat = ones_f.rearrange("p a b c -> p (a b c)")
                    for _d in range(14):
                        psd = psmm.tile([128, 512], f32, tag="ps")
                        nc.tensor.matmul(
                            psd[:, 0:224], ones_flat[:, 0:128], ones_flat,
                            start=True, stop=True,
                        )
                    # first x block + wkT first, split small: DMA transfers
                    # are the startup critical path
                    xblk0 = xs.tile([128, 8, 256], bf16, tag="xblk")
                    wkTr = wkT.rearrange("(j p) m -> p j m", p=128)
                    nc.sync.dma_start(xblk0[:, 0:4], xTr[:, 0:4, 0:256])
                    nc.scalar.dma_start(wkT_sb[:, 0:4], wkTr[:, 0:4])
                    nc.sync.dma_start(xblk0[:, 4:8], xTr[:, 4:8, 0:256])
                    nc.scalar.dma_start(wkT_sb[:, 4:8], wkTr[:, 4:8])
                    # prefetch block 1's x ahead of qT/wvT: block-1 kT would
                    # otherwise starve on the serial DMA chain
                    xblk1 = xs.tile([128, 8, 256], bf16, tag="xblk")
                    nc.sync.dma_start(xblk1, xTr[:, :, 256:512])
                    nc.gpsimd.dma_start(qT_sb, qT.ap())
                    nc.scalar.dma_start(
                        wvT_sb, wvT.rearrange("(j p) m -> p j m", p=128))
                    for bi in range(6):
                        kv_block(bi, psmm)
                        for j in range(NPAIR):
                            qk_block(bi, j, psst, "stA", "stB")
                    kv_block(6, psmm)

                # ---- last block's QK + AV + normalize + proj mt0/mt1 ----
                # (kv pool closed: the QK slots + AV accumulators fill all 8
                # banks; AV sweeps interleave with the block-6 exp tail)
                with tc.tile_pool(name="psout", bufs=1, space="PSUM") as psout:
                    qk_block(6, 0, psout, "stA6", "stB6")
                    qk_block(6, 1, psout, "stA6", "stB6")
                    av_pair(0, psout)
                    qk_block(6, 2, psout, "stA6", "stB6")
                    av_pair(1, psout)
                    qk_block(6, 3, psout, "stA6", "stB6")
                    av_pair(2, psout)
                    av_pair(3, psout)
                    # proj row-tiles mt0/mt1 reuse the QK slots (same size);
                    # jj 0-2 fill the last normalization chain's PE gap
                    pp0 = psout.tile([128, C], f32, tag="stA6")
                    pp1 = psout.tile([128, C], f32, tag="stB6")
                    for jj in range(4):
                        proj_pair(pp0, pp1, jj)
                    for mi, ps in ((0, pp0), (1, pp1)):
                        yt = yp.tile([128, C], bf16, tag="yt")
                        if mi == 0:
                            nc.scalar.copy(yt, ps)
                        else:
                            nc.vector.tensor_copy(yt, ps)
                        nc.sync.dma_start(y.ap()[mi], yt)

            # ---- remaining projection row-tiles (wk/xs/kts closed) ----
            with (
                tc.tile_pool(name="pspj2", bufs=2, space="PSUM") as pspj2,
                tc.tile_pool(name="yp2", bufs=3) as yp2,
            ):
                for mt in range(2, 5):
                    m0 = mt * 128
                    mcols = 66 if mt == 4 else 128   # lhsT free width (even)
                    mrows = 65 if mt == 4 else 128   # valid output rows
                    ps = pspj2.tile([128, C], f32, tag="pp")
                    for jj in range(4):
                        for (c0, c1) in ((0, 512), (512, C)):
                            nc.tensor.matmul(
                                ps[:mcols, c0:c1],
                                outTb[:, jj, m0:m0 + mcols],
                                wpT_sb[:, jj, c0:c1],
                                start=(jj == 0), stop=(jj == 3),
                            )
                    yt = yp2.tile([128, C], bf16, tag="yt")
                    if mt == 3:
                        nc.vector.tensor_copy(yt[:mrows], ps[:mrows])
                    else:
                        nc.scalar.copy(yt[:mrows], ps[:mrows])
                    nc.sync.dma_start(y.ap()[mt, 0:mrows], yt[:mrows])

    nc.finalize()
    return nc


_NC_CACHE = None


def _get_nc():
    global _NC_CACHE
    if _NC_CACHE is None:
        _NC_CACHE = _build_nc()
    return _NC_CACHE


def _host_prep(x, q_learned, pos_embed, Wk, Wv, Wp, rpe_W, rp_bucket):
    """Build the 8 per-core input maps."""
    bf = ml_dtypes.bfloat16
    x = np.asarray(x, dtype=np.float32)
    q_ = (np.asarray(q_learned, np.float32) + np.asarray(pos_embed, np.float32))[0]
    Wk = np.asarray(Wk, np.float32)
    Wv = np.asarray(Wv, np.float32)
    Wp = np.asarray(Wp, np.float32)
    rpe_W = np.asarray(rpe_W, np.float32)
    rp_bucket = np.asarray(rp_bucket)

    scale = HD ** -0.5

    # RPE bias, expanded to key-tile layout: [H, n, q] -> pair-major
    qh = q_.reshape(LQ, H, HD)
    rpe_tab = np.einsum('qhd,dn->hqn', qh, rpe_W)                  # (H, LQ, nb)
    rpe = np.take_along_axis(
        rpe_tab, np.broadcast_to(rp_bucket[None], (H, LQ, LQ)), axis=-1
    )                                                              # (H, q, j')
    n_idx = np.arange(NPAD)
    jcol = np.where(n_idx == 0, 0, 1 + (n_idx - 1) % P_SP)         # (NPAD,)
    biasx = rpe[:, :, jcol]                                        # (H, q, n)
    biasx[:, :, NKV:] = 0.0
    biasx = np.exp(biasx)                # multiplicative bias: exp(S)*exp(b)
    biasx = np.ascontiguousarray(biasx.transpose(0, 2, 1))         # (H, n, q)
    # (H=8 per group, n, q) -> [pair, t, p, hsel, q]
    biasg = biasx.reshape(2, NPAIR, 2, NT, 128, LQ).transpose(0, 1, 3, 4, 2, 5)

    # qT per group, scaled, padded: (2, 128, 4, QF)
    qTg = np.zeros((2, 512, QF), np.float32)
    qTg[:, :, :LQ] = (q_.T * scale).reshape(2, 512, LQ)
    qTg = qTg.reshape(2, 4, 128, QF).transpose(0, 2, 1, 3)

    per_group = []
    for g in range(2):
        sl = slice(g * 512, (g + 1) * 512)
        per_group.append({
            "wkT": np.ascontiguousarray(Wk[sl, :].T).astype(bf),
            "wvT": np.ascontiguousarray(Wv[sl, :].T).astype(bf),
            "wpT": np.ascontiguousarray(Wp[:, sl].T).astype(bf),
            "qT": np.ascontiguousarray(qTg[g]).astype(bf),
            "biasx": np.ascontiguousarray(biasg[g]).astype(bf),
        })

    in_maps = []
    for b in range(B):
        xTb = np.zeros((C, NPAD), bf)
        xTb[:, :NKV] = x[b].T.astype(bf)
        for g in range(2):
            m = dict(per_group[g])
            m["xT"] = xTb
            in_maps.append(m)
    return in_maps


def kernel(x, q_learned, pos_embed, Wk, Wv, Wp, bp, rpe_W, rp_bucket):
    in_maps = _host_prep(x, q_learned, pos_embed, Wk, Wv, Wp, rpe_W, rp_bucket)
    nc = _get_nc()

    last_err = None
    for _attempt in range(3):
        try:
            res = run_bass_kernel_spmd(nc, in_maps, core_ids=list(range(NCORES)))
            break
        except Exception as e:  # wedged-device recovery: retry
            last_err = e
    else:
        raise last_err

    bp = np.asarray(bp, np.float32)
    out = np.empty((B, LQ, C), np.float32)
    for b in range(B):
        y0 = res.results[2 * b]["y"].reshape(640, C)[:LQ].astype(np.float32)
        y1 = res.results[2 * b + 1]["y"].reshape(640, C)[:LQ].astype(np.float32)
        out[b] = y0 + y1 + bp
    return out
